# revision 23
# baseline (speedup 1.0000x reference)
"""Trainium2 Bass kernel v4 for nn_M04AdaptiveVQ.

Data-parallel over B: each of 8 NeuronCores handles one batch element.

vs v3:
- VQ scan: no per-chunk bias matmuls -- |c|^2 and a GLOBAL 13-bit index
  iota are folded into a host-built f32 `combo` table added by one DVE
  scalar_tensor_tensor after a single scalar-engine +2^23 rounding op
  (was: bias MM + 2 scalar adds + DVE TT + max8 per chunk).
- Rescore identical (exact fp32 top-6 via gathered rows + stt dots).
- lin_dec gathers batched into ONE 1536-index gather each (frame-major
  bf16 with dec_b2 pre-folded; c-major bf16 for the MLP inputs).
- RVQ: no bias matmuls (|cb|^2 is negligible vs 2r.cb); scores converted
  to bf16 by the scalar engine, then MAX8 + MAX_INDEX give the argmax
  directly (no f32 pack/extract); one batched 1536-index code gather per
  stage; fused full-T qs/r updates in bf16 at 2x DVE rate.
- Decoder output: dec_b2 folded into the gathered lin_dec table; final
  add fused into one stt (psum*scale + lin_dec) per 128x512 tile.
"""
import sys
if '/opt/trn_rl_repo' not in sys.path:
    sys.path.insert(0, '/opt/trn_rl_repo')
import numpy as np
import ml_dtypes

B, C, T = 8, 1024, 1500
K = 8192
D, H = 256, 512
NQ, BINS = 4, 1024
TPAD = 1536
NFT = TPAD // 128
NKC = K // 512
NCC = C // 128
NCP = C // 256
NDC = D // 128
NHC = H // 128
NTT = TPAD // 512
EPS = 1e-8
CSHIFT = 2304.0
MASKI = 0x1FFF
NCAND = 5
WS = 32.0
SCALES = {"enc": (1.0, 1.0), "nrm": (4.0, 8.0), "dec": (16.0, 16.0)}

TRACE = False
DEBUG = False
SIM_RELU = False
LAST_RESULT = None
_prog = None


def _build_program():
    import concourse.bacc as bacc
    import concourse.mybir as mybir
    from concourse.tile import TileContext
    from contextlib import ExitStack

    f32 = mybir.dt.float32
    bf16 = mybir.dt.bfloat16
    f8 = mybir.dt.float8e4
    u32 = mybir.dt.uint32
    u16 = mybir.dt.uint16
    i16 = mybir.dt.int16
    AF = mybir.ActivationFunctionType
    OP = mybir.AluOpType
    AX = mybir.AxisListType
    PM = mybir.MatmulPerfMode

    nc = bacc.Bacc("TRN2", target_bir_lowering=False, debug=False, num_devices=8)

    dfeat8 = nc.dram_tensor("feat8", [128, NCP, 2, TPAD], f8, kind="ExternalInput")
    dfeat = nc.dram_tensor("feat", [C, TPAD], f32, kind="ExternalInput")
    dfeatT = nc.dram_tensor("featT", [TPAD, C], f32, kind="ExternalInput")
    dcent8 = nc.dram_tensor("cent8", [128, NCP, 2, K], f8, kind="ExternalInput")
    dcombo = nc.dram_tensor("combo", [128, K], f32, kind="ExternalInput")
    dcent_ext = nc.dram_tensor("cent_ext", [K, 1088], f32, kind="ExternalInput")
    dcent_ob = nc.dram_tensor("cent_ob", [K, C], bf16, kind="ExternalInput")
    dcent16 = nc.dram_tensor("cent16", [K, C], bf16, kind="ExternalInput")
    dw = {}
    for pre, shapes in (
        ("enc", [(C, H), (H, H), (H, D)]),
        ("nrm", [(C, H), (H, H), (H, D)]),
        ("dec", [(D, H), (H, H), (H, C)]),
    ):
        for i, (ni, no) in enumerate(shapes):
            dw[f"{pre}_w{i}"] = nc.dram_tensor(
                f"{pre}_w8{i}", [128, ni // 256, 2, no], f8, kind="ExternalInput")
            dw[f"{pre}_b{i}"] = nc.dram_tensor(
                f"{pre}_bs{i}", [no], f32, kind="ExternalInput")
    dcbT2 = nc.dram_tensor("cbT2", [NQ, D, BINS], bf16, kind="ExternalInput")
    dcb16 = nc.dram_tensor("cb16", [NQ, BINS, D], bf16, kind="ExternalInput")
    dout = nc.dram_tensor("out", [TPAD, C], f32, kind="ExternalOutput")
    if DEBUG:
        ddbg_sel = nc.dram_tensor("dbg_sel", [128, NFT], u16, kind="ExternalOutput")
        ddbg_rsel = nc.dram_tensor("dbg_rsel", [NQ, 128, NFT], u16,
                                   kind="ExternalOutput")

    def wrap_cols(idxw, src16, ncols):
        # idxw: (128, ncols, 8) i16; dest [q, a, b] = src16[16*b + q, a]
        for b in range(8):
            nc.sync.dma_start(out=idxw[0:16, :, b],
                              in_=src16[16 * b:16 * b + 16, :].bitcast(i16))
        for g in range(1, 8):
            nc.sync.dma_start(out=idxw[16 * g:16 * g + 16], in_=idxw[0:16])

    with TileContext(nc) as tc:
        with ExitStack() as top:
            const = top.enter_context(tc.tile_pool(name="const", bufs=1))
            maski8 = const.tile([128, 8], u32)
            nc.vector.memset(maski8[:], MASKI)
            eps_col = const.tile([128, 1], f32)
            nc.vector.memset(eps_col[:], EPS)
            neg2 = const.tile([128, 1], f32)
            nc.vector.memset(neg2[:], -2.0)
            bigc = const.tile([128, 1], f32)
            nc.vector.memset(bigc[:], 2.0 ** 23)
            nbigc = const.tile([128, 1], f32)
            nc.vector.memset(nbigc[:], -(2.0 ** 23))

            idxgp = top.enter_context(tc.tile_pool(name="idxgp", bufs=1))
            idxwA = idxgp.tile([128, NFT, 8], i16)

            # ============ Phase A: VQ scan + rescore + lin_dec ============
            with tc.tile_pool(name="centp", bufs=1) as centp, \
                 tc.tile_pool(name="xTp", bufs=2) as xTp, \
                 tc.tile_pool(name="gathp", bufs=2) as gathp, \
                 tc.tile_pool(name="t1p", bufs=4) as t1p, \
                 tc.tile_pool(name="t2p", bufs=1) as t2p, \
                 tc.tile_pool(name="pkp", bufs=2) as pkp, \
                 tc.tile_pool(name="candp", bufs=2) as candp, \
                 tc.tile_pool(name="idxp", bufs=2) as idxp, \
                 tc.tile_pool(name="scrp", bufs=1) as scrp, \
                 tc.tile_pool(name="sps", bufs=8, space="PSUM") as sps:
                cent8 = centp.tile([128, NCP, 2, K], f8)
                nc.sync.dma_start(out=cent8[:], in_=dcent8[:])
                combo = centp.tile([128, K], f32)
                nc.sync.dma_start(out=combo[:], in_=dcombo[:])
                feat8 = centp.tile([128, NCP, 2, TPAD], f8)
                nc.sync.dma_start(out=feat8[:], in_=dfeat8[:])

                selA = candp.tile([128, NFT], f32, tag="selA")

                def rescore(ctx):
                    ft, xT, g5, candf = ctx
                    dmat = candp.tile([128, 8], f32, tag="dmat")
                    nc.vector.memset(dmat[:], 3.0e38)
                    for a in range(NCAND):
                        scr = scrp.tile([128, C], f32, tag="scr")
                        nc.vector.scalar_tensor_tensor(
                            out=scr[:], in0=g5[:, a, 0:C], scalar=neg2[:],
                            in1=xT[:], op0=OP.mult, op1=OP.mult,
                            accum_out=dmat[:, a:a + 1])
                        nc.vector.tensor_tensor(out=dmat[:, a:a + 1],
                                                in0=dmat[:, a:a + 1],
                                                in1=g5[:, a, 1024:1025],
                                                op=OP.add)
                    dmin = candp.tile([128, 1], f32, tag="dmin")
                    nc.vector.tensor_reduce(dmin[:], dmat[:], axis=AX.X,
                                            op=OP.min)
                    msk = candp.tile([128, 8], f32, tag="msk")
                    nc.vector.tensor_scalar(msk[:], dmat[:], dmin[:], None,
                                            op0=OP.is_le)
                    nc.vector.tensor_tensor(out=msk[:], in0=msk[:],
                                            in1=candf[:], op=OP.mult)
                    dsel = candp.tile([128, 1], f32, tag="dsel")
                    nc.vector.tensor_reduce(dsel[:], msk[:], axis=AX.X,
                                            op=OP.max)
                    nc.vector.tensor_copy(selA[:, ft:ft + 1], dsel[:])
                    if DEBUG:
                        sel16d = candp.tile([128, 1], u16, tag="sel16d")
                        nc.vector.tensor_copy(sel16d[:], dsel[:])
                        nc.sync.dma_start(out=ddbg_sel[:, ft:ft + 1],
                                          in_=sel16d[:])

                prev = None
                for ft in range(NFT):
                    fs = slice(ft * 128, (ft + 1) * 128)
                    xT = xTp.tile([128, C], f32, tag="xT")
                    nc.sync.dma_start(out=xT[:], in_=dfeatT[fs, :])

                    pkg = pkp.tile([128, 4 * 8], f32, tag="pkg")
                    for blk in range(4):
                        t1 = t1p.tile([128, 4, 512], f32, tag="t1")
                        for j in range(4):
                            kc = 4 * blk + j
                            ks = slice(kc * 512, (kc + 1) * 512)
                            ps = sps.tile([128, 512], f32, tag="sps")
                            for cp in range(NCP):
                                nc.tensor.matmul(ps[:], feat8[:, cp, :, fs],
                                                 cent8[:, cp, :, ks],
                                                 start=(cp == 0),
                                                 stop=(cp == NCP - 1),
                                                 perf_mode=PM.DoubleRow,
                                                 skip_group_check=True)
                            nc.scalar.add(t1[:, j], ps[:], bigc[:])
                        t2 = t2p.tile([128, 2048], f32, tag="t2")
                        bs_ = slice(blk * 2048, (blk + 1) * 2048)
                        nc.vector.scalar_tensor_tensor(
                            out=t2[:], in0=t1[:].rearrange("p a b -> p (a b)"),
                            scalar=nbigc[:], in1=combo[:, bs_],
                            op0=OP.add, op1=OP.add)
                        nc.vector.max(out=pkg[:, blk * 8:blk * 8 + 8],
                                      in_=t2[:])
                    g8 = candp.tile([128, 8], f32, tag="g8")
                    nc.vector.max(out=g8[:], in_=pkg[:])
                    cand = candp.tile([128, 8], u32, tag="cand")
                    nc.vector.tensor_tensor(out=cand[:], in0=g8[:].bitcast(u32),
                                            in1=maski8[:], op=OP.bitwise_and)
                    candf = candp.tile([128, 8], f32, tag="candf")
                    nc.vector.tensor_copy(candf[:], cand[:])
                    cand16 = candp.tile([128, 8], u16, tag="cand16")
                    nc.vector.tensor_copy(cand16[:], cand[:])
                    idxw6 = idxp.tile([128, NCAND, 8], i16, tag="idxw6")
                    wrap_cols(idxw6, cand16[:, 0:NCAND], NCAND)
                    g5 = gathp.tile([128, NCAND, 1088], f32, tag="g5")
                    nc.gpsimd.dma_gather(
                        out_ap=g5[:], in_ap=dcent_ext[:],
                        idxs_ap=idxw6[:],
                        num_idxs=NCAND * 128, num_idxs_reg=NCAND * 128,
                        elem_size=1088)
                    if prev is not None:
                        rescore(prev)
                    prev = (ft, xT, g5, candf)
                rescore(prev)

                selA16 = candp.tile([128, NFT], u16, tag="selA16")
                nc.vector.tensor_copy(selA16[:], selA[:])
                wrap_cols(idxwA, selA16[:], NFT)

            # ---- batched lin_dec gathers (512 idx each, tt-aligned) ----
            ld16p = top.enter_context(tc.tile_pool(name="ld16p", bufs=1))
            ld16s = [ld16p.tile([128, NCC, 512], bf16, tag=f"ld16_{tt}",
                                name=f"ld16_{tt}") for tt in range(NTT)]
            linFp = top.enter_context(tc.tile_pool(name="linFp", bufs=1))
            linF = linFp.tile([128, NFT, C], bf16)     # lin_dec+b2 frame-major
            for tt in range(NTT):
                nc.gpsimd.dma_gather(
                    out_ap=linF[:, 4 * tt:4 * tt + 4], in_ap=dcent_ob[:],
                    idxs_ap=idxwA[:, 4 * tt:4 * tt + 4], num_idxs=512,
                    num_idxs_reg=512, elem_size=C)
                nc.gpsimd.dma_gather(
                    out_ap=ld16s[tt][:], in_ap=dcent16[:],
                    idxs_ap=idxwA[:, 4 * tt:4 * tt + 4], num_idxs=512,
                    num_idxs_reg=512, elem_size=C, transpose=True)

            # ============ weights + RVQ tables ============
            wp = top.enter_context(tc.tile_pool(name="wp", bufs=1))
            w8 = {}
            bs = {}
            for pre, shapes in (("enc", [(C, H), (H, H), (H, D)]),
                                ("nrm", [(C, H), (H, H), (H, D)]),
                                ("dec", [(D, H), (H, H), (H, C)])):
                for i, (ni, no) in enumerate(shapes):
                    t = wp.tile([128, ni // 256, 2, no], f8, tag=f"w{pre}{i}")
                    nc.sync.dma_start(out=t[:], in_=dw[f"{pre}_w{i}"][:])
                    w8[f"{pre}{i}"] = t
                    bt = wp.tile([128, no // 128], f32, tag=f"b{pre}{i}")
                    nc.sync.dma_start(
                        out=bt[:],
                        in_=dw[f"{pre}_b{i}"][:].rearrange("(a p) -> p a", p=128))
                    bs[f"{pre}{i}"] = bt

            rqp = top.enter_context(tc.tile_pool(name="rqp", bufs=1))
            r_t = rqp.tile([128, NDC, TPAD], bf16)
            qs = rqp.tile([128, NDC, TPAD], bf16)
            nc.vector.memset(qs[:], 0.0)
            nvp = top.enter_context(tc.tile_pool(name="nvp", bufs=1))
            norm_v = nvp.tile([128, NDC, TPAD], f32)

            frep = top.enter_context(tc.tile_pool(name="frep", bufs=2))
            s8p = top.enter_context(tc.tile_pool(name="s8p", bufs=1))
            hp = top.enter_context(tc.tile_pool(name="hp", bufs=2))
            sep = top.enter_context(tc.tile_pool(name="sep", bufs=1))
            mps = top.enter_context(tc.tile_pool(name="mps", bufs=4, space="PSUM"))
            rps = top.enter_context(tc.tile_pool(name="rps", bufs=4, space="PSUM"))
            scp = top.enter_context(tc.tile_pool(name="scp", bufs=2))
            packr = top.enter_context(tc.tile_pool(name="packr", bufs=2))
            gqp = top.enter_context(tc.tile_pool(name="gqp", bufs=2))
            cbtp = top.enter_context(tc.tile_pool(name="cbtp", bufs=2))
            idxr = top.enter_context(tc.tile_pool(name="idxr", bufs=2))
            rselp = top.enter_context(tc.tile_pool(name="rselp", bufs=2))
            sedp = top.enter_context(tc.tile_pool(name="sedp", bufs=2))
            outp = top.enter_context(tc.tile_pool(name="outp", bufs=2))

            def mlp_fp8(pre, in8, out_f32, htag=""):
                s1, s2 = SCALES[pre]
                shapes = {"enc": [(C, H), (H, H), (H, D)],
                          "nrm": [(C, H), (H, H), (H, D)],
                          "dec": [(D, H), (H, H), (H, C)]}[pre]
                LR = AF.Relu if SIM_RELU else AF.Lrelu
                funcs = {"enc": (LR, LR, AF.Identity),
                         "nrm": (AF.Relu, AF.Relu, AF.Relu),
                         "dec": (LR, LR, None)}[pre]
                alphas = {"enc": 0.01, "dec": 0.01, "nrm": 0.0}[pre]
                scl = [s1 / WS, s2 / (s1 * WS), 1.0 / (s2 * WS)]
                cur = in8
                for li, (ni, no) in enumerate(shapes):
                    npair = ni // 256
                    last = (li == 2)
                    if not last:
                        h = hp.tile([128, no // 256, 2, 512], f8,
                                    tag=f"h{pre}{li}{htag}")
                    for oc in range(no // 128):
                        ps = mps.tile([128, 512], f32, tag="mlp_ps")
                        for cp in range(npair):
                            nc.tensor.matmul(
                                ps[:],
                                w8[f"{pre}{li}"][:, cp, :, oc * 128:(oc + 1) * 128],
                                cur[:, cp], start=(cp == 0),
                                stop=(cp == npair - 1),
                                perf_mode=PM.DoubleRow, skip_group_check=True)
                        if last:
                            nc.scalar.activation(
                                out_f32[:, oc], ps[:], funcs[2] or AF.Identity,
                                bias=bs[f"{pre}{li}"][:, oc:oc + 1],
                                scale=scl[2], alpha=alphas)
                        else:
                            nc.scalar.activation(
                                h[:, oc // 2, oc % 2], ps[:], funcs[li],
                                bias=bs[f"{pre}{li}"][:, oc:oc + 1],
                                scale=scl[li], alpha=alphas)
                    cur = h

            # ============ Phase B: enc + nrm + sen per tt ============
            for tt in range(NTT):
                ts_ = slice(tt * 512, (tt + 1) * 512)
                spk8 = s8p.tile([128, NCP, 2, 512], f8, tag="spk8")
                for cc in range(NCC):
                    fre = frep.tile([128, 512], f32, tag="fre")
                    nc.sync.dma_start(out=fre[:],
                                      in_=dfeat[cc * 128:(cc + 1) * 128, ts_])
                    nc.vector.tensor_tensor(
                        out=spk8[:, cc // 2, cc % 2],
                        in0=fre[:], in1=ld16s[tt][:, cc], op=OP.subtract)
                spk_enc = sep.tile([128, NDC, 512], f32, tag="spk_enc")
                mlp_fp8("enc", spk8, spk_enc)
                nrm8 = s8p.tile([128, NCP, 2, 512], f8, tag="nrm8")
                nc.vector.tensor_copy(
                    nrm8[:].rearrange("p cp i f -> p (cp i) f"),
                    ld16s[tt][:])
                mlp_fp8("nrm", nrm8, norm_v[:, :, ts_])
                rec = sep.tile([128, NDC, 512], f32, tag="rec")
                nc.scalar.add(rec[:], norm_v[:, :, ts_], eps_col[:])
                nc.vector.reciprocal_approx_fast(rec[:], rec[:])
                nc.vector.tensor_tensor(out=r_t[:, :, ts_], in0=spk_enc[:],
                                        in1=rec[:], op=OP.mult)

            # ============ Phase C: RVQ stage-outer ============
            for q in range(NQ):
                cbt = cbtp.tile([128, NDC, BINS], bf16, tag="cbt")
                nc.sync.dma_start(
                    out=cbt[:],
                    in_=dcbT2[q].rearrange("(a p) n -> p a n", p=128))
                rsel = rselp.tile([128, NFT], u16, tag="rsel")
                idxw12 = idxr.tile([128, NFT, 8], i16, tag="idxw12")

                def rvq_update(ctx, q=q):
                    tt, gq16 = ctx
                    ts_ = slice(tt * 512, (tt + 1) * 512)
                    nc.vector.tensor_tensor(out=qs[:, :, ts_],
                                            in0=qs[:, :, ts_],
                                            in1=gq16[:], op=OP.add)
                    if q < NQ - 1:
                        nc.vector.tensor_tensor(out=r_t[:, :, ts_],
                                                in0=r_t[:, :, ts_],
                                                in1=gq16[:], op=OP.subtract)

                pend = None
                for tt in range(NTT):
                    for ftl in range(4):
                        ft = 4 * tt + ftl
                        fs = slice(ft * 128, (ft + 1) * 128)
                        sc = scp.tile([128, BINS], bf16, tag="sc")
                        for half in range(2):
                            hs = slice(half * 512, (half + 1) * 512)
                            psq = rps.tile([128, 512], f32, tag="psq")
                            for dc in range(NDC):
                                nc.tensor.matmul(psq[:], r_t[:, dc, fs],
                                                 cbt[:, dc, hs],
                                                 start=(dc == 0),
                                                 stop=(dc == NDC - 1),
                                                 skip_group_check=True)
                            nc.scalar.activation(
                                sc[:, hs], psq[:], AF.Identity)
                        m8 = packr.tile([128, 8], bf16, tag="m8")
                        nc.vector.max(out=m8[:], in_=sc[:])
                        mi = packr.tile([128, 8], u16, tag="mi")
                        nc.vector.max_index(mi[:], m8[:], sc[:])
                        nc.vector.tensor_copy(rsel[:, ft:ft + 1], mi[:, 0:1])
                    wrap_cols(idxw12[:, 4 * tt:4 * tt + 4],
                              rsel[:, 4 * tt:4 * tt + 4], 4)
                    gq16 = gqp.tile([128, NDC, 512], bf16, tag="gq16")
                    nc.gpsimd.dma_gather(
                        out_ap=gq16[:], in_ap=dcb16[q],
                        idxs_ap=idxw12[:, 4 * tt:4 * tt + 4], num_idxs=512,
                        num_idxs_reg=512, elem_size=D, transpose=True)
                    if pend is not None:
                        rvq_update(pend)
                    pend = (tt, gq16)
                rvq_update(pend)
                if DEBUG:
                    nc.sync.dma_start(out=ddbg_rsel[q], in_=rsel[:])

            # ============ Phase D: sed + dec + out per tt ============
            s1, s2 = SCALES["dec"]
            for tt in range(NTT):
                ts_ = slice(tt * 512, (tt + 1) * 512)
                sed8 = sedp.tile([128, 1, 2, 512], f8, tag="sed8")
                nc.vector.tensor_tensor(out=sed8[:, 0], in0=qs[:, :, ts_],
                                        in1=norm_v[:, :, ts_], op=OP.mult)
                hd0 = hp.tile([128, 2, 2, 512], f8, tag="hd0")
                for oc in range(NHC):
                    ps = mps.tile([128, 512], f32, tag="mlp_ps")
                    nc.tensor.matmul(ps[:],
                                     w8["dec0"][:, 0, :, oc * 128:(oc + 1) * 128],
                                     sed8[:, 0], start=True, stop=True,
                                     perf_mode=PM.DoubleRow,
                                     skip_group_check=True)
                    nc.scalar.activation(hd0[:, oc // 2, oc % 2], ps[:],
                                         AF.Relu if SIM_RELU else AF.Lrelu,
                                         bias=bs["dec0"][:, oc:oc + 1],
                                         scale=s1 / WS, alpha=0.01)
                hd1 = hp.tile([128, 2, 2, 512], f8, tag="hd1")
                for oc in range(NHC):
                    ps = mps.tile([128, 512], f32, tag="mlp_ps")
                    for cp in range(2):
                        nc.tensor.matmul(
                            ps[:], w8["dec1"][:, cp, :, oc * 128:(oc + 1) * 128],
                            hd0[:, cp], start=(cp == 0), stop=(cp == 1),
                            perf_mode=PM.DoubleRow, skip_group_check=True)
                    nc.scalar.activation(hd1[:, oc // 2, oc % 2], ps[:],
                                         AF.Relu if SIM_RELU else AF.Lrelu,
                                         bias=bs["dec1"][:, oc:oc + 1],
                                         scale=s2 / (s1 * WS), alpha=0.01)
                for ftl in range(4):
                    ft = tt * 4 + ftl
                    fs = slice(ft * 128, (ft + 1) * 128)
                    fsl = slice(ftl * 128, (ftl + 1) * 128)
                    for half in range(2):
                        cs = slice(half * 512, (half + 1) * 512)
                        ps = mps.tile([128, 512], f32, tag="mlp_ps")
                        for cp in range(2):
                            nc.tensor.matmul(
                                ps[:], hd1[:, cp, :, fsl],
                                w8["dec2"][:, cp, :, cs],
                                start=(cp == 0), stop=(cp == 1),
                                perf_mode=PM.DoubleRow, skip_group_check=True)
                        ot = outp.tile([128, 512], f32, tag="ot")
                        nc.vector.scalar_tensor_tensor(
                            out=ot[:], in0=ps[:], scalar=1.0 / (s2 * WS),
                            in1=linF[:, ft, cs], op0=OP.mult, op1=OP.add)
                        nc.sync.dma_start(out=dout[fs, cs], in_=ot[:])

    nc.compile()
    return nc


def _get_program():
    global _prog
    if _prog is None:
        _prog = _build_program()
    return _prog


def _host_prep(inputs):
    fp8 = ml_dtypes.float8_e4m3fn
    bf = ml_dtypes.bfloat16
    g = lambda k: np.ascontiguousarray(np.asarray(inputs[k], dtype=np.float32))
    feature = g('feature')
    centroid = g('centroid')
    codebooks = g('codebooks')

    cent_pair = np.ascontiguousarray(
        (2.0 * centroid.T).reshape(NCP, 2, 128, K).transpose(2, 0, 1, 3)
    ).astype(fp8)
    cn = (centroid.astype(np.float64) ** 2).sum(1)
    cent_ext = np.zeros((K, 1088), dtype=np.float32)
    cent_ext[:, :C] = centroid
    cent_ext[:, C] = cn.astype(np.float32)
    combo_row = (np.round(CSHIFT - cn) + np.arange(K) * 2.0 ** -13
                 ).astype(np.float32)
    combo = np.ascontiguousarray(np.broadcast_to(combo_row[None, :], (128, K)))

    shared = {
        "cent8": cent_pair,
        "combo": combo,
        "cent_ext": cent_ext,
        "cent_ob": (centroid + g("dec_b2")[None, :]).astype(bf),
        "cent16": centroid.astype(bf),
        "cbT2": np.ascontiguousarray(
            2.0 * codebooks.transpose(0, 2, 1)).astype(bf),
        "cb16": codebooks.astype(bf),
    }
    for pre in ("enc", "nrm", "dec"):
        s1, s2 = SCALES[pre]
        bscale = {0: s1, 1: s2, 2: 1.0}
        for i in range(3):
            w = g(f"{pre}_w{i}")
            ni, no = w.shape
            shared[f"{pre}_w8{i}"] = np.ascontiguousarray(
                (w * WS).reshape(ni // 256, 2, 128, no).transpose(2, 0, 1, 3)
            ).astype(fp8)
            shared[f"{pre}_bs{i}"] = np.ascontiguousarray(
                g(f"{pre}_b{i}") * bscale[i])

    in_maps = []
    for b in range(B):
        m = dict(shared)
        feats = np.zeros((C, TPAD), dtype=np.float32)
        feats[:, :T] = feature[b]
        m["feat"] = feats
        m["featT"] = np.ascontiguousarray(feats.T)
        m["feat8"] = np.ascontiguousarray(
            feats.reshape(NCP, 2, 128, TPAD).transpose(2, 0, 1, 3)).astype(fp8)
        in_maps.append(m)
    return in_maps


def kernel(**inputs):
    global LAST_RESULT
    from concourse.bass_utils import run_bass_kernel_spmd
    nc = _get_program()
    in_maps = _host_prep(inputs)
    kwargs = {}
    if TRACE:
        try:
            from ntff_shim import install_ntff_hook
            install_ntff_hook()
            kwargs["trace"] = True
        except Exception:
            pass
    res = run_bass_kernel_spmd(nc, in_maps, core_ids=list(range(B)), **kwargs)
    LAST_RESULT = res
    out = np.empty((B, C, T), dtype=np.float32)
    for b in range(B):
        out[b] = res.results[b]["out"][:T].T
    return out


# revision 26
# speedup vs baseline: 1.0632x; 1.0632x over previous
"""Trainium2 Bass kernel v4 for nn_M04AdaptiveVQ.

Data-parallel over B: each of 8 NeuronCores handles one batch element.

vs v3:
- VQ scan: no per-chunk bias matmuls -- |c|^2 and a GLOBAL 13-bit index
  iota are folded into a host-built f32 `combo` table added by one DVE
  scalar_tensor_tensor after a single scalar-engine +2^23 rounding op
  (was: bias MM + 2 scalar adds + DVE TT + max8 per chunk).
- Rescore identical (exact fp32 top-6 via gathered rows + stt dots).
- lin_dec gathers batched into ONE 1536-index gather each (frame-major
  bf16 with dec_b2 pre-folded; c-major bf16 for the MLP inputs).
- RVQ: no bias matmuls (|cb|^2 is negligible vs 2r.cb); scores converted
  to bf16 by the scalar engine, then MAX8 + MAX_INDEX give the argmax
  directly (no f32 pack/extract); one batched 1536-index code gather per
  stage; fused full-T qs/r updates in bf16 at 2x DVE rate.
- Decoder output: dec_b2 folded into the gathered lin_dec table; final
  add fused into one stt (psum*scale + lin_dec) per 128x512 tile.
"""
import sys
if '/opt/trn_rl_repo' not in sys.path:
    sys.path.insert(0, '/opt/trn_rl_repo')
import numpy as np
import ml_dtypes

B, C, T = 8, 1024, 1500
K = 8192
D, H = 256, 512
NQ, BINS = 4, 1024
TPAD = 1536
NFT = TPAD // 128
NKC = K // 512
NCC = C // 128
NCP = C // 256
NDC = D // 128
NHC = H // 128
NTT = TPAD // 512
EPS = 1e-8
CSHIFT = 2304.0
MASKI = 0x1FFF
NCAND = 5
WS = 32.0
SCALES = {"enc": (1.0, 1.0), "nrm": (4.0, 8.0), "dec": (16.0, 16.0)}

TRACE = False
DEBUG = False
SIM_RELU = False
LAST_RESULT = None
_prog = None


def _build_program():
    import concourse.bacc as bacc
    import concourse.mybir as mybir
    from concourse.tile import TileContext
    from contextlib import ExitStack

    f32 = mybir.dt.float32
    bf16 = mybir.dt.bfloat16
    f8 = mybir.dt.float8e4
    u32 = mybir.dt.uint32
    u16 = mybir.dt.uint16
    i16 = mybir.dt.int16
    AF = mybir.ActivationFunctionType
    OP = mybir.AluOpType
    AX = mybir.AxisListType
    PM = mybir.MatmulPerfMode

    nc = bacc.Bacc("TRN2", target_bir_lowering=False, debug=False, num_devices=8)

    dfeat8 = nc.dram_tensor("feat8", [128, NCP, 2, TPAD], f8, kind="ExternalInput")
    dfeat = nc.dram_tensor("feat", [C, TPAD], f32, kind="ExternalInput")
    dfeatT = nc.dram_tensor("featT", [TPAD, C], f32, kind="ExternalInput")
    dcent8 = nc.dram_tensor("cent8", [128, NCP, 2, K], f8, kind="ExternalInput")
    dcombo = nc.dram_tensor("combo", [128, K], f32, kind="ExternalInput")
    dcent_ext = nc.dram_tensor("cent_ext", [K, 1088], f32, kind="ExternalInput")
    dcent_ob = nc.dram_tensor("cent_ob", [K, C], bf16, kind="ExternalInput")
    dcent16 = nc.dram_tensor("cent16", [K, C], bf16, kind="ExternalInput")
    dw = {}
    for pre, shapes in (
        ("enc", [(C, H), (H, H), (H, D)]),
        ("nrm", [(C, H), (H, H), (H, D)]),
        ("dec", [(D, H), (H, H), (H, C)]),
    ):
        for i, (ni, no) in enumerate(shapes):
            dw[f"{pre}_w{i}"] = nc.dram_tensor(
                f"{pre}_w8{i}", [128, ni // 256, 2, no], f8, kind="ExternalInput")
            dw[f"{pre}_b{i}"] = nc.dram_tensor(
                f"{pre}_bs{i}", [no], f32, kind="ExternalInput")
    dcbT2 = nc.dram_tensor("cbT2", [NQ, D, BINS], bf16, kind="ExternalInput")
    dcb16 = nc.dram_tensor("cb16", [NQ, BINS, D], bf16, kind="ExternalInput")
    dout = nc.dram_tensor("out", [TPAD, C], f32, kind="ExternalOutput")
    if DEBUG:
        ddbg_sel = nc.dram_tensor("dbg_sel", [128, NFT], u16, kind="ExternalOutput")
        ddbg_rsel = nc.dram_tensor("dbg_rsel", [NQ, 128, NFT], u16,
                                   kind="ExternalOutput")

    def wrap_cols(idxw, src16, ncols):
        # idxw: (128, ncols, 8) i16; dest [q, a, b] = src16[16*b + q, a]
        for b in range(8):
            nc.sync.dma_start(out=idxw[0:16, :, b],
                              in_=src16[16 * b:16 * b + 16, :].bitcast(i16))
        for g in range(1, 8):
            nc.sync.dma_start(out=idxw[16 * g:16 * g + 16], in_=idxw[0:16])

    with TileContext(nc) as tc:
        with ExitStack() as top:
            const = top.enter_context(tc.tile_pool(name="const", bufs=1))
            maski8 = const.tile([128, 8], u32)
            nc.vector.memset(maski8[:], MASKI)
            eps_col = const.tile([128, 1], f32)
            nc.vector.memset(eps_col[:], EPS)
            neg2 = const.tile([128, 1], f32)
            nc.vector.memset(neg2[:], -2.0)
            bigc = const.tile([128, 1], f32)
            nc.vector.memset(bigc[:], 2.0 ** 23)
            nbigc = const.tile([128, 1], f32)
            nc.vector.memset(nbigc[:], -(2.0 ** 23))

            idxgp = top.enter_context(tc.tile_pool(name="idxgp", bufs=1))
            idxwA = idxgp.tile([128, NFT, 8], i16)

            # ============ Phase A: VQ scan + rescore + lin_dec ============
            with tc.tile_pool(name="centp", bufs=1) as centp, \
                 tc.tile_pool(name="xTp", bufs=2) as xTp, \
                 tc.tile_pool(name="gathp", bufs=2) as gathp, \
                 tc.tile_pool(name="t1p", bufs=4) as t1p, \
                 tc.tile_pool(name="t2p", bufs=1) as t2p, \
                 tc.tile_pool(name="pkp", bufs=2) as pkp, \
                 tc.tile_pool(name="candp", bufs=2) as candp, \
                 tc.tile_pool(name="idxp", bufs=2) as idxp, \
                 tc.tile_pool(name="scrp", bufs=1) as scrp, \
                 tc.tile_pool(name="sps", bufs=8, space="PSUM") as sps:
                cent8 = centp.tile([128, NCP, 2, K], f8)
                nc.sync.dma_start(out=cent8[:], in_=dcent8[:])
                combo = centp.tile([128, K], f32)
                nc.sync.dma_start(out=combo[:], in_=dcombo[:])
                feat8 = centp.tile([128, NCP, 2, TPAD], f8)
                nc.sync.dma_start(out=feat8[:], in_=dfeat8[:])

                selA = candp.tile([128, NFT], f32, tag="selA")

                def rescore(ctx):
                    ft, xT, g5, candf = ctx
                    dmat = candp.tile([128, NCAND], f32, tag="dmat")
                    for a in range(NCAND):
                        scr = scrp.tile([128, C], f32, tag="scr")
                        nc.vector.scalar_tensor_tensor(
                            out=scr[:], in0=g5[:, a, 0:C], scalar=neg2[:],
                            in1=xT[:], op0=OP.mult, op1=OP.mult,
                            accum_out=dmat[:, a:a + 1])
                    nc.vector.tensor_tensor(out=dmat[:], in0=dmat[:],
                                            in1=g5[:, :, 1024], op=OP.add)
                    dmin = candp.tile([128, 1], f32, tag="dmin")
                    nc.vector.tensor_reduce(dmin[:], dmat[:], axis=AX.X,
                                            op=OP.min)
                    msk = candp.tile([128, NCAND], f32, tag="msk")
                    nc.vector.tensor_scalar(msk[:], dmat[:], dmin[:], None,
                                            op0=OP.is_le)
                    nc.vector.tensor_tensor(out=msk[:], in0=msk[:],
                                            in1=candf[:, 0:NCAND],
                                            op=OP.mult)
                    dsel = candp.tile([128, 1], f32, tag="dsel")
                    nc.vector.tensor_reduce(dsel[:], msk[:], axis=AX.X,
                                            op=OP.max)
                    nc.vector.tensor_copy(selA[:, ft:ft + 1], dsel[:])
                    if DEBUG:
                        sel16d = candp.tile([128, 1], u16, tag="sel16d")
                        nc.vector.tensor_copy(sel16d[:], dsel[:])
                        nc.sync.dma_start(out=ddbg_sel[:, ft:ft + 1],
                                          in_=sel16d[:])

                prev = None
                for ft in range(NFT):
                    fs = slice(ft * 128, (ft + 1) * 128)
                    xT = xTp.tile([128, C], f32, tag="xT")
                    nc.sync.dma_start(out=xT[:], in_=dfeatT[fs, :])

                    pkg = pkp.tile([128, 4 * 8], f32, tag="pkg")
                    for blk in range(4):
                        t1 = t1p.tile([128, 4, 512], f32, tag="t1")
                        for j in range(4):
                            kc = 4 * blk + j
                            ks = slice(kc * 512, (kc + 1) * 512)
                            ps = sps.tile([128, 512], f32, tag="sps")
                            for cp in range(NCP):
                                nc.tensor.matmul(ps[:], feat8[:, cp, :, fs],
                                                 cent8[:, cp, :, ks],
                                                 start=(cp == 0),
                                                 stop=(cp == NCP - 1),
                                                 perf_mode=PM.DoubleRow,
                                                 skip_group_check=True)
                            nc.scalar.add(t1[:, j], ps[:], bigc[:])
                        t2 = t2p.tile([128, 2048], f32, tag="t2")
                        bs_ = slice(blk * 2048, (blk + 1) * 2048)
                        nc.vector.scalar_tensor_tensor(
                            out=t2[:], in0=t1[:].rearrange("p a b -> p (a b)"),
                            scalar=nbigc[:], in1=combo[:, bs_],
                            op0=OP.add, op1=OP.add)
                        nc.vector.max(out=pkg[:, blk * 8:blk * 8 + 8],
                                      in_=t2[:])
                    g8 = candp.tile([128, 8], f32, tag="g8")
                    nc.vector.max(out=g8[:], in_=pkg[:])
                    cand = candp.tile([128, 8], u32, tag="cand")
                    nc.vector.tensor_tensor(out=cand[:], in0=g8[:].bitcast(u32),
                                            in1=maski8[:], op=OP.bitwise_and)
                    candf = candp.tile([128, 8], f32, tag="candf")
                    nc.vector.tensor_copy(candf[:], cand[:])
                    cand16 = candp.tile([128, 8], u16, tag="cand16")
                    nc.vector.tensor_copy(cand16[:], cand[:])
                    idxw6 = idxp.tile([128, NCAND, 8], i16, tag="idxw6")
                    wrap_cols(idxw6, cand16[:, 0:NCAND], NCAND)
                    g5 = gathp.tile([128, NCAND, 1088], f32, tag="g5")
                    nc.gpsimd.dma_gather(
                        out_ap=g5[:], in_ap=dcent_ext[:],
                        idxs_ap=idxw6[:],
                        num_idxs=NCAND * 128, num_idxs_reg=NCAND * 128,
                        elem_size=1088)
                    if prev is not None:
                        rescore(prev)
                    prev = (ft, xT, g5, candf)
                rescore(prev)

                selA16 = candp.tile([128, NFT], u16, tag="selA16")
                nc.vector.tensor_copy(selA16[:], selA[:])
                wrap_cols(idxwA, selA16[:], NFT)

            # ---- batched lin_dec gathers (512 idx each, tt-aligned) ----
            ld16p = top.enter_context(tc.tile_pool(name="ld16p", bufs=1))
            ld16s = [ld16p.tile([128, NCC, 512], bf16, tag=f"ld16_{tt}",
                                name=f"ld16_{tt}") for tt in range(NTT)]
            linFp = top.enter_context(tc.tile_pool(name="linFp", bufs=1))
            linF = linFp.tile([128, NFT, C], bf16)     # lin_dec+b2 frame-major
            for tt in range(NTT):
                nc.gpsimd.dma_gather(
                    out_ap=linF[:, 4 * tt:4 * tt + 4], in_ap=dcent_ob[:],
                    idxs_ap=idxwA[:, 4 * tt:4 * tt + 4], num_idxs=512,
                    num_idxs_reg=512, elem_size=C)
                nc.gpsimd.dma_gather(
                    out_ap=ld16s[tt][:], in_ap=dcent16[:],
                    idxs_ap=idxwA[:, 4 * tt:4 * tt + 4], num_idxs=512,
                    num_idxs_reg=512, elem_size=C, transpose=True)

            # ============ weights + RVQ tables ============
            wp = top.enter_context(tc.tile_pool(name="wp", bufs=1))
            w8 = {}
            bs = {}
            for pre, shapes in (("enc", [(C, H), (H, H), (H, D)]),
                                ("nrm", [(C, H), (H, H), (H, D)]),
                                ("dec", [(D, H), (H, H), (H, C)])):
                for i, (ni, no) in enumerate(shapes):
                    t = wp.tile([128, ni // 256, 2, no], f8, tag=f"w{pre}{i}")
                    nc.sync.dma_start(out=t[:], in_=dw[f"{pre}_w{i}"][:])
                    w8[f"{pre}{i}"] = t
                    bt = wp.tile([128, no // 128], f32, tag=f"b{pre}{i}")
                    nc.sync.dma_start(
                        out=bt[:],
                        in_=dw[f"{pre}_b{i}"][:].rearrange("(a p) -> p a", p=128))
                    bs[f"{pre}{i}"] = bt

            rqp = top.enter_context(tc.tile_pool(name="rqp", bufs=1))
            r_t = rqp.tile([128, NDC, TPAD], bf16)
            qs = rqp.tile([128, NDC, TPAD], bf16)
            nc.vector.memset(qs[:], 0.0)
            nvp = top.enter_context(tc.tile_pool(name="nvp", bufs=1))
            norm_v = nvp.tile([128, NDC, TPAD], f32)

            frep = top.enter_context(tc.tile_pool(name="frep", bufs=2))
            s8p = top.enter_context(tc.tile_pool(name="s8p", bufs=1))
            hp = top.enter_context(tc.tile_pool(name="hp", bufs=2))
            sep = top.enter_context(tc.tile_pool(name="sep", bufs=1))
            mps = top.enter_context(tc.tile_pool(name="mps", bufs=4, space="PSUM"))
            rps = top.enter_context(tc.tile_pool(name="rps", bufs=4, space="PSUM"))
            scp = top.enter_context(tc.tile_pool(name="scp", bufs=2))
            packr = top.enter_context(tc.tile_pool(name="packr", bufs=2))
            gqp = top.enter_context(tc.tile_pool(name="gqp", bufs=2))
            cbtp = top.enter_context(tc.tile_pool(name="cbtp", bufs=2))
            idxr = top.enter_context(tc.tile_pool(name="idxr", bufs=2))
            rselp = top.enter_context(tc.tile_pool(name="rselp", bufs=2))
            sedp = top.enter_context(tc.tile_pool(name="sedp", bufs=2))
            outp = top.enter_context(tc.tile_pool(name="outp", bufs=2))

            def mlp_fp8(pre, in8, out_f32, htag=""):
                s1, s2 = SCALES[pre]
                shapes = {"enc": [(C, H), (H, H), (H, D)],
                          "nrm": [(C, H), (H, H), (H, D)],
                          "dec": [(D, H), (H, H), (H, C)]}[pre]
                LR = AF.Relu if SIM_RELU else AF.Lrelu
                funcs = {"enc": (LR, LR, AF.Identity),
                         "nrm": (AF.Relu, AF.Relu, AF.Relu),
                         "dec": (LR, LR, None)}[pre]
                alphas = {"enc": 0.01, "dec": 0.01, "nrm": 0.0}[pre]
                scl = [s1 / WS, s2 / (s1 * WS), 1.0 / (s2 * WS)]
                cur = in8
                for li, (ni, no) in enumerate(shapes):
                    npair = ni // 256
                    last = (li == 2)
                    if not last:
                        h = hp.tile([128, no // 256, 2, 512], f8,
                                    tag=f"h{pre}{li}{htag}")
                    for oc in range(no // 128):
                        ps = mps.tile([128, 512], f32, tag="mlp_ps")
                        for cp in range(npair):
                            nc.tensor.matmul(
                                ps[:],
                                w8[f"{pre}{li}"][:, cp, :, oc * 128:(oc + 1) * 128],
                                cur[:, cp], start=(cp == 0),
                                stop=(cp == npair - 1),
                                perf_mode=PM.DoubleRow, skip_group_check=True)
                        if last:
                            nc.scalar.activation(
                                out_f32[:, oc], ps[:], funcs[2] or AF.Identity,
                                bias=bs[f"{pre}{li}"][:, oc:oc + 1],
                                scale=scl[2], alpha=alphas)
                        else:
                            nc.scalar.activation(
                                h[:, oc // 2, oc % 2], ps[:], funcs[li],
                                bias=bs[f"{pre}{li}"][:, oc:oc + 1],
                                scale=scl[li], alpha=alphas)
                    cur = h

            # ============ Phase B: enc + nrm + sen per tt ============
            for tt in range(NTT):
                ts_ = slice(tt * 512, (tt + 1) * 512)
                spk8 = s8p.tile([128, NCP, 2, 512], f8, tag="spk8")
                for cc in range(NCC):
                    fre = frep.tile([128, 512], f32, tag="fre")
                    nc.sync.dma_start(out=fre[:],
                                      in_=dfeat[cc * 128:(cc + 1) * 128, ts_])
                    nc.vector.tensor_tensor(
                        out=spk8[:, cc // 2, cc % 2],
                        in0=fre[:], in1=ld16s[tt][:, cc], op=OP.subtract)
                spk_enc = sep.tile([128, NDC, 512], f32, tag="spk_enc")
                mlp_fp8("enc", spk8, spk_enc)
                nrm8 = s8p.tile([128, NCP, 2, 512], f8, tag="nrm8")
                nc.vector.tensor_copy(
                    nrm8[:].rearrange("p cp i f -> p (cp i) f"),
                    ld16s[tt][:])
                mlp_fp8("nrm", nrm8, norm_v[:, :, ts_])
                rec = sep.tile([128, NDC, 512], f32, tag="rec")
                nc.scalar.add(rec[:], norm_v[:, :, ts_], eps_col[:])
                nc.vector.reciprocal_approx_fast(rec[:], rec[:])
                nc.vector.tensor_tensor(out=r_t[:, :, ts_], in0=spk_enc[:],
                                        in1=rec[:], op=OP.mult)

            # ============ Phase C: RVQ stage-outer ============
            for q in range(NQ):
                cbt = cbtp.tile([128, NDC, BINS], bf16, tag="cbt")
                nc.sync.dma_start(
                    out=cbt[:],
                    in_=dcbT2[q].rearrange("(a p) n -> p a n", p=128))
                rsel = rselp.tile([128, NFT], u16, tag="rsel")
                idxw12 = idxr.tile([128, NFT, 8], i16, tag="idxw12")

                def rvq_update(ctx, q=q):
                    tt, gq16 = ctx
                    ts_ = slice(tt * 512, (tt + 1) * 512)
                    nc.vector.tensor_tensor(out=qs[:, :, ts_],
                                            in0=qs[:, :, ts_],
                                            in1=gq16[:], op=OP.add)
                    if q < NQ - 1:
                        nc.vector.tensor_tensor(out=r_t[:, :, ts_],
                                                in0=r_t[:, :, ts_],
                                                in1=gq16[:], op=OP.subtract)

                pend = None
                for tt in range(NTT):
                    for ftl in range(4):
                        ft = 4 * tt + ftl
                        fs = slice(ft * 128, (ft + 1) * 128)
                        sc = scp.tile([128, BINS], bf16, tag="sc")
                        for half in range(2):
                            hs = slice(half * 512, (half + 1) * 512)
                            psq = rps.tile([128, 512], f32, tag="psq")
                            for dc in range(NDC):
                                nc.tensor.matmul(psq[:], r_t[:, dc, fs],
                                                 cbt[:, dc, hs],
                                                 start=(dc == 0),
                                                 stop=(dc == NDC - 1),
                                                 skip_group_check=True)
                            nc.scalar.activation(
                                sc[:, hs], psq[:], AF.Identity)
                        m8 = packr.tile([128, 8], bf16, tag="m8")
                        nc.vector.max(out=m8[:], in_=sc[:])
                        mi = packr.tile([128, 8], u16, tag="mi")
                        nc.vector.max_index(mi[:], m8[:], sc[:])
                        nc.vector.tensor_copy(rsel[:, ft:ft + 1], mi[:, 0:1])
                    wrap_cols(idxw12[:, 4 * tt:4 * tt + 4],
                              rsel[:, 4 * tt:4 * tt + 4], 4)
                    gq16 = gqp.tile([128, NDC, 512], bf16, tag="gq16")
                    nc.gpsimd.dma_gather(
                        out_ap=gq16[:], in_ap=dcb16[q],
                        idxs_ap=idxw12[:, 4 * tt:4 * tt + 4], num_idxs=512,
                        num_idxs_reg=512, elem_size=D, transpose=True)
                    if pend is not None:
                        rvq_update(pend)
                    pend = (tt, gq16)
                rvq_update(pend)
                if DEBUG:
                    nc.sync.dma_start(out=ddbg_rsel[q], in_=rsel[:])

            # ============ Phase D: sed + dec + out per tt ============
            s1, s2 = SCALES["dec"]
            for tt in range(NTT):
                ts_ = slice(tt * 512, (tt + 1) * 512)
                sed8 = sedp.tile([128, 1, 2, 512], f8, tag="sed8")
                nc.vector.tensor_tensor(out=sed8[:, 0], in0=qs[:, :, ts_],
                                        in1=norm_v[:, :, ts_], op=OP.mult)
                hd0 = hp.tile([128, 2, 2, 512], f8, tag="hd0")
                for oc in range(NHC):
                    ps = mps.tile([128, 512], f32, tag="mlp_ps")
                    nc.tensor.matmul(ps[:],
                                     w8["dec0"][:, 0, :, oc * 128:(oc + 1) * 128],
                                     sed8[:, 0], start=True, stop=True,
                                     perf_mode=PM.DoubleRow,
                                     skip_group_check=True)
                    nc.scalar.activation(hd0[:, oc // 2, oc % 2], ps[:],
                                         AF.Relu if SIM_RELU else AF.Lrelu,
                                         bias=bs["dec0"][:, oc:oc + 1],
                                         scale=s1 / WS, alpha=0.01)
                hd1 = hp.tile([128, 2, 2, 512], f8, tag="hd1")
                for oc in range(NHC):
                    ps = mps.tile([128, 512], f32, tag="mlp_ps")
                    for cp in range(2):
                        nc.tensor.matmul(
                            ps[:], w8["dec1"][:, cp, :, oc * 128:(oc + 1) * 128],
                            hd0[:, cp], start=(cp == 0), stop=(cp == 1),
                            perf_mode=PM.DoubleRow, skip_group_check=True)
                    nc.scalar.activation(hd1[:, oc // 2, oc % 2], ps[:],
                                         AF.Relu if SIM_RELU else AF.Lrelu,
                                         bias=bs["dec1"][:, oc:oc + 1],
                                         scale=s2 / (s1 * WS), alpha=0.01)
                for ftl in range(4):
                    ft = tt * 4 + ftl
                    fs = slice(ft * 128, (ft + 1) * 128)
                    fsl = slice(ftl * 128, (ftl + 1) * 128)
                    for half in range(2):
                        cs = slice(half * 512, (half + 1) * 512)
                        ps = mps.tile([128, 512], f32, tag="mlp_ps")
                        for cp in range(2):
                            nc.tensor.matmul(
                                ps[:], hd1[:, cp, :, fsl],
                                w8["dec2"][:, cp, :, cs],
                                start=(cp == 0), stop=(cp == 1),
                                perf_mode=PM.DoubleRow, skip_group_check=True)
                        ot = outp.tile([128, 512], f32, tag="ot")
                        nc.vector.scalar_tensor_tensor(
                            out=ot[:], in0=ps[:], scalar=1.0 / (s2 * WS),
                            in1=linF[:, ft, cs], op0=OP.mult, op1=OP.add)
                        nc.sync.dma_start(out=dout[fs, cs], in_=ot[:])

    nc.compile()
    return nc


def _get_program():
    global _prog
    if _prog is None:
        _prog = _build_program()
    return _prog


def _host_prep(inputs):
    fp8 = ml_dtypes.float8_e4m3fn
    bf = ml_dtypes.bfloat16
    g = lambda k: np.ascontiguousarray(np.asarray(inputs[k], dtype=np.float32))
    feature = g('feature')
    centroid = g('centroid')
    codebooks = g('codebooks')

    cent_pair = np.ascontiguousarray(
        (2.0 * centroid.T).reshape(NCP, 2, 128, K).transpose(2, 0, 1, 3)
    ).astype(fp8)
    cn = (centroid.astype(np.float64) ** 2).sum(1)
    cent_ext = np.zeros((K, 1088), dtype=np.float32)
    cent_ext[:, :C] = centroid
    cent_ext[:, C] = cn.astype(np.float32)
    combo_row = (np.round(CSHIFT - cn) + np.arange(K) * 2.0 ** -13
                 ).astype(np.float32)
    combo = np.ascontiguousarray(np.broadcast_to(combo_row[None, :], (128, K)))

    shared = {
        "cent8": cent_pair,
        "combo": combo,
        "cent_ext": cent_ext,
        "cent_ob": (centroid + g("dec_b2")[None, :]).astype(bf),
        "cent16": centroid.astype(bf),
        "cbT2": np.ascontiguousarray(
            2.0 * codebooks.transpose(0, 2, 1)).astype(bf),
        "cb16": codebooks.astype(bf),
    }
    for pre in ("enc", "nrm", "dec"):
        s1, s2 = SCALES[pre]
        bscale = {0: s1, 1: s2, 2: 1.0}
        for i in range(3):
            w = g(f"{pre}_w{i}")
            ni, no = w.shape
            shared[f"{pre}_w8{i}"] = np.ascontiguousarray(
                (w * WS).reshape(ni // 256, 2, 128, no).transpose(2, 0, 1, 3)
            ).astype(fp8)
            shared[f"{pre}_bs{i}"] = np.ascontiguousarray(
                g(f"{pre}_b{i}") * bscale[i])

    in_maps = []
    for b in range(B):
        m = dict(shared)
        feats = np.zeros((C, TPAD), dtype=np.float32)
        feats[:, :T] = feature[b]
        m["feat"] = feats
        m["featT"] = np.ascontiguousarray(feats.T)
        m["feat8"] = np.ascontiguousarray(
            feats.reshape(NCP, 2, 128, TPAD).transpose(2, 0, 1, 3)).astype(fp8)
        in_maps.append(m)
    return in_maps


def kernel(**inputs):
    global LAST_RESULT
    from concourse.bass_utils import run_bass_kernel_spmd
    nc = _get_program()
    in_maps = _host_prep(inputs)
    kwargs = {}
    if TRACE:
        try:
            from ntff_shim import install_ntff_hook
            install_ntff_hook()
            kwargs["trace"] = True
        except Exception:
            pass
    res = run_bass_kernel_spmd(nc, in_maps, core_ids=list(range(B)), **kwargs)
    LAST_RESULT = res
    out = np.empty((B, C, T), dtype=np.float32)
    for b in range(B):
        out[b] = res.results[b]["out"][:T].T
    return out


# revision 27
# speedup vs baseline: 1.0853x; 1.0208x over previous
"""Trainium2 Bass kernel v4 for nn_M04AdaptiveVQ.

Data-parallel over B: each of 8 NeuronCores handles one batch element.

vs v3:
- VQ scan: no per-chunk bias matmuls -- |c|^2 and a GLOBAL 13-bit index
  iota are folded into a host-built f32 `combo` table added by one DVE
  scalar_tensor_tensor after a single scalar-engine +2^23 rounding op
  (was: bias MM + 2 scalar adds + DVE TT + max8 per chunk).
- Rescore identical (exact fp32 top-6 via gathered rows + stt dots).
- lin_dec gathers batched into ONE 1536-index gather each (frame-major
  bf16 with dec_b2 pre-folded; c-major bf16 for the MLP inputs).
- RVQ: no bias matmuls (|cb|^2 is negligible vs 2r.cb); scores converted
  to bf16 by the scalar engine, then MAX8 + MAX_INDEX give the argmax
  directly (no f32 pack/extract); one batched 1536-index code gather per
  stage; fused full-T qs/r updates in bf16 at 2x DVE rate.
- Decoder output: dec_b2 folded into the gathered lin_dec table; final
  add fused into one stt (psum*scale + lin_dec) per 128x512 tile.
"""
import sys
if '/opt/trn_rl_repo' not in sys.path:
    sys.path.insert(0, '/opt/trn_rl_repo')
import numpy as np
import ml_dtypes

B, C, T = 8, 1024, 1500
K = 8192
D, H = 256, 512
NQ, BINS = 4, 1024
TPAD = 1536
NFT = TPAD // 128
NKC = K // 512
NCC = C // 128
NCP = C // 256
NDC = D // 128
NHC = H // 128
NTT = TPAD // 512
EPS = 1e-8
CSHIFT = 2304.0
MASKI = 0x1FFF
NCAND = 5
WS = 32.0
SCALES = {"enc": (1.0, 1.0), "nrm": (4.0, 8.0), "dec": (16.0, 16.0)}

TRACE = False
DEBUG = False
SIM_RELU = False
LAST_RESULT = None
_prog = None


def _build_program():
    import concourse.bacc as bacc
    import concourse.mybir as mybir
    from concourse.tile import TileContext
    from contextlib import ExitStack

    f32 = mybir.dt.float32
    bf16 = mybir.dt.bfloat16
    f8 = mybir.dt.float8e4
    u32 = mybir.dt.uint32
    u16 = mybir.dt.uint16
    i16 = mybir.dt.int16
    AF = mybir.ActivationFunctionType
    OP = mybir.AluOpType
    AX = mybir.AxisListType
    PM = mybir.MatmulPerfMode

    nc = bacc.Bacc("TRN2", target_bir_lowering=False, debug=False, num_devices=8)

    dfeat8 = nc.dram_tensor("feat8", [128, NCP, 2, TPAD], f8, kind="ExternalInput")
    dfeat = nc.dram_tensor("feat", [C, TPAD], f32, kind="ExternalInput")
    dfeatT = nc.dram_tensor("featT", [TPAD, C], f32, kind="ExternalInput")
    dcent8 = nc.dram_tensor("cent8", [128, NCP, 2, K], f8, kind="ExternalInput")
    dcombo = nc.dram_tensor("combo", [128, K], f32, kind="ExternalInput")
    dcent_ext = nc.dram_tensor("cent_ext", [K, 1088], f32, kind="ExternalInput")
    dcent_ob = nc.dram_tensor("cent_ob", [K, C], bf16, kind="ExternalInput")
    dcent16 = nc.dram_tensor("cent16", [K, C], bf16, kind="ExternalInput")
    dw = {}
    for pre, shapes in (
        ("enc", [(C, H), (H, H), (H, D)]),
        ("nrm", [(C, H), (H, H), (H, D)]),
        ("dec", [(D, H), (H, H), (H, C)]),
    ):
        for i, (ni, no) in enumerate(shapes):
            dw[f"{pre}_w{i}"] = nc.dram_tensor(
                f"{pre}_w8{i}", [128, ni // 256, 2, no], f8, kind="ExternalInput")
            dw[f"{pre}_b{i}"] = nc.dram_tensor(
                f"{pre}_bs{i}", [no], f32, kind="ExternalInput")
    dcbT2 = nc.dram_tensor("cbT2", [NQ, D, BINS], bf16, kind="ExternalInput")
    dcb16 = nc.dram_tensor("cb16", [NQ, BINS, D], bf16, kind="ExternalInput")
    dout = nc.dram_tensor("out", [TPAD, C], f32, kind="ExternalOutput")
    if DEBUG:
        ddbg_sel = nc.dram_tensor("dbg_sel", [128, NFT], u16, kind="ExternalOutput")
        ddbg_rsel = nc.dram_tensor("dbg_rsel", [NQ, 128, NFT], u16,
                                   kind="ExternalOutput")

    def wrap_cols(idxw, src16, ncols):
        # idxw: (128, ncols, 8) i16; dest [q, a, b] = src16[16*b + q, a]
        for b in range(8):
            nc.sync.dma_start(out=idxw[0:16, :, b],
                              in_=src16[16 * b:16 * b + 16, :].bitcast(i16))
        # replicate to all 128 partitions by log-doubling
        nc.sync.dma_start(out=idxw[16:32], in_=idxw[0:16])
        nc.sync.dma_start(out=idxw[32:64], in_=idxw[0:32])
        nc.sync.dma_start(out=idxw[64:128], in_=idxw[0:64])

    with TileContext(nc) as tc:
        with ExitStack() as top:
            const = top.enter_context(tc.tile_pool(name="const", bufs=1))
            maski8 = const.tile([128, 8], u32)
            nc.vector.memset(maski8[:], MASKI)
            eps_col = const.tile([128, 1], f32)
            nc.vector.memset(eps_col[:], EPS)
            neg2 = const.tile([128, 1], f32)
            nc.vector.memset(neg2[:], -2.0)
            bigc = const.tile([128, 1], f32)
            nc.vector.memset(bigc[:], 2.0 ** 23)
            nbigc = const.tile([128, 1], f32)
            nc.vector.memset(nbigc[:], -(2.0 ** 23))

            idxgp = top.enter_context(tc.tile_pool(name="idxgp", bufs=1))
            idxwA = idxgp.tile([128, NFT, 8], i16)

            # ============ Phase A: VQ scan + rescore + lin_dec ============
            with tc.tile_pool(name="centp", bufs=1) as centp, \
                 tc.tile_pool(name="xTp", bufs=2) as xTp, \
                 tc.tile_pool(name="gathp", bufs=2) as gathp, \
                 tc.tile_pool(name="t1p", bufs=4) as t1p, \
                 tc.tile_pool(name="t2p", bufs=1) as t2p, \
                 tc.tile_pool(name="pkp", bufs=2) as pkp, \
                 tc.tile_pool(name="candp", bufs=2) as candp, \
                 tc.tile_pool(name="idxp", bufs=2) as idxp, \
                 tc.tile_pool(name="scrp", bufs=1) as scrp, \
                 tc.tile_pool(name="sps", bufs=8, space="PSUM") as sps:
                cent8 = centp.tile([128, NCP, 2, K], f8)
                nc.sync.dma_start(out=cent8[:], in_=dcent8[:])
                combo = centp.tile([128, K], f32)
                nc.sync.dma_start(out=combo[:], in_=dcombo[:])
                feat8 = centp.tile([128, NCP, 2, TPAD], f8)
                nc.sync.dma_start(out=feat8[:], in_=dfeat8[:])

                selA = candp.tile([128, NFT], f32, tag="selA")

                def rescore(ctx):
                    ft, xT, g5, candf = ctx
                    dmat = candp.tile([128, NCAND], f32, tag="dmat")
                    for a in range(NCAND):
                        scr = scrp.tile([128, C], f32, tag="scr")
                        nc.vector.scalar_tensor_tensor(
                            out=scr[:], in0=g5[:, a, 0:C], scalar=neg2[:],
                            in1=xT[:], op0=OP.mult, op1=OP.mult,
                            accum_out=dmat[:, a:a + 1])
                    nc.vector.tensor_tensor(out=dmat[:], in0=dmat[:],
                                            in1=g5[:, :, 1024], op=OP.add)
                    dmin = candp.tile([128, 1], f32, tag="dmin")
                    nc.vector.tensor_reduce(dmin[:], dmat[:], axis=AX.X,
                                            op=OP.min)
                    msk = candp.tile([128, NCAND], f32, tag="msk")
                    nc.vector.tensor_scalar(msk[:], dmat[:], dmin[:], None,
                                            op0=OP.is_le)
                    nc.vector.tensor_tensor(out=msk[:], in0=msk[:],
                                            in1=candf[:, 0:NCAND],
                                            op=OP.mult)
                    dsel = candp.tile([128, 1], f32, tag="dsel")
                    nc.vector.tensor_reduce(dsel[:], msk[:], axis=AX.X,
                                            op=OP.max)
                    nc.vector.tensor_copy(selA[:, ft:ft + 1], dsel[:])
                    if DEBUG:
                        sel16d = candp.tile([128, 1], u16, tag="sel16d")
                        nc.vector.tensor_copy(sel16d[:], dsel[:])
                        nc.sync.dma_start(out=ddbg_sel[:, ft:ft + 1],
                                          in_=sel16d[:])

                prev = None
                for ft in range(NFT):
                    fs = slice(ft * 128, (ft + 1) * 128)
                    xT = xTp.tile([128, C], f32, tag="xT")
                    nc.sync.dma_start(out=xT[:], in_=dfeatT[fs, :])

                    pkg = pkp.tile([128, 4 * 8], f32, tag="pkg")
                    for blk in range(4):
                        t1 = t1p.tile([128, 4, 512], f32, tag="t1")
                        for j in range(4):
                            kc = 4 * blk + j
                            ks = slice(kc * 512, (kc + 1) * 512)
                            ps = sps.tile([128, 512], f32, tag="sps")
                            for cp in range(NCP):
                                nc.tensor.matmul(ps[:], feat8[:, cp, :, fs],
                                                 cent8[:, cp, :, ks],
                                                 start=(cp == 0),
                                                 stop=(cp == NCP - 1),
                                                 perf_mode=PM.DoubleRow,
                                                 skip_group_check=True)
                            nc.scalar.add(t1[:, j], ps[:], bigc[:])
                        t2 = t2p.tile([128, 2048], f32, tag="t2")
                        bs_ = slice(blk * 2048, (blk + 1) * 2048)
                        nc.vector.scalar_tensor_tensor(
                            out=t2[:], in0=t1[:].rearrange("p a b -> p (a b)"),
                            scalar=nbigc[:], in1=combo[:, bs_],
                            op0=OP.add, op1=OP.add)
                        nc.vector.max(out=pkg[:, blk * 8:blk * 8 + 8],
                                      in_=t2[:])
                    g8 = candp.tile([128, 8], f32, tag="g8")
                    nc.vector.max(out=g8[:], in_=pkg[:])
                    cand = candp.tile([128, 8], u32, tag="cand")
                    nc.vector.tensor_tensor(out=cand[:], in0=g8[:].bitcast(u32),
                                            in1=maski8[:], op=OP.bitwise_and)
                    candf = candp.tile([128, 8], f32, tag="candf")
                    nc.vector.tensor_copy(candf[:], cand[:])
                    cand16 = candp.tile([128, 8], u16, tag="cand16")
                    nc.vector.tensor_copy(cand16[:], cand[:])
                    idxw6 = idxp.tile([128, NCAND, 8], i16, tag="idxw6")
                    wrap_cols(idxw6, cand16[:, 0:NCAND], NCAND)
                    g5 = gathp.tile([128, NCAND, 1088], f32, tag="g5")
                    nc.gpsimd.dma_gather(
                        out_ap=g5[:], in_ap=dcent_ext[:],
                        idxs_ap=idxw6[:],
                        num_idxs=NCAND * 128, num_idxs_reg=NCAND * 128,
                        elem_size=1088)
                    if prev is not None:
                        rescore(prev)
                    prev = (ft, xT, g5, candf)
                rescore(prev)

                selA16 = candp.tile([128, NFT], u16, tag="selA16")
                nc.vector.tensor_copy(selA16[:], selA[:])
                wrap_cols(idxwA, selA16[:], NFT)

            # ---- batched lin_dec gathers (512 idx each, tt-aligned) ----
            ld16p = top.enter_context(tc.tile_pool(name="ld16p", bufs=1))
            ld16s = [ld16p.tile([128, NCC, 512], bf16, tag=f"ld16_{tt}",
                                name=f"ld16_{tt}") for tt in range(NTT)]
            linFp = top.enter_context(tc.tile_pool(name="linFp", bufs=1))
            linF = linFp.tile([128, NFT, C], bf16)     # lin_dec+b2 frame-major
            for tt in range(NTT):
                nc.gpsimd.dma_gather(
                    out_ap=linF[:, 4 * tt:4 * tt + 4], in_ap=dcent_ob[:],
                    idxs_ap=idxwA[:, 4 * tt:4 * tt + 4], num_idxs=512,
                    num_idxs_reg=512, elem_size=C)
                nc.gpsimd.dma_gather(
                    out_ap=ld16s[tt][:], in_ap=dcent16[:],
                    idxs_ap=idxwA[:, 4 * tt:4 * tt + 4], num_idxs=512,
                    num_idxs_reg=512, elem_size=C, transpose=True)

            # ============ weights + RVQ tables ============
            wp = top.enter_context(tc.tile_pool(name="wp", bufs=1))
            w8 = {}
            bs = {}
            for pre, shapes in (("enc", [(C, H), (H, H), (H, D)]),
                                ("nrm", [(C, H), (H, H), (H, D)]),
                                ("dec", [(D, H), (H, H), (H, C)])):
                for i, (ni, no) in enumerate(shapes):
                    t = wp.tile([128, ni // 256, 2, no], f8, tag=f"w{pre}{i}")
                    nc.sync.dma_start(out=t[:], in_=dw[f"{pre}_w{i}"][:])
                    w8[f"{pre}{i}"] = t
                    bt = wp.tile([128, no // 128], f32, tag=f"b{pre}{i}")
                    nc.sync.dma_start(
                        out=bt[:],
                        in_=dw[f"{pre}_b{i}"][:].rearrange("(a p) -> p a", p=128))
                    bs[f"{pre}{i}"] = bt

            rqp = top.enter_context(tc.tile_pool(name="rqp", bufs=1))
            r_t = rqp.tile([128, NDC, TPAD], bf16)
            qs = rqp.tile([128, NDC, TPAD], bf16)
            nc.vector.memset(qs[:], 0.0)
            nvp = top.enter_context(tc.tile_pool(name="nvp", bufs=1))
            norm_v = nvp.tile([128, NDC, TPAD], f32)

            frep = top.enter_context(tc.tile_pool(name="frep", bufs=2))
            s8p = top.enter_context(tc.tile_pool(name="s8p", bufs=1))
            hp = top.enter_context(tc.tile_pool(name="hp", bufs=2))
            sep = top.enter_context(tc.tile_pool(name="sep", bufs=1))
            mps = top.enter_context(tc.tile_pool(name="mps", bufs=4, space="PSUM"))
            rps = top.enter_context(tc.tile_pool(name="rps", bufs=4, space="PSUM"))
            scp = top.enter_context(tc.tile_pool(name="scp", bufs=2))
            packr = top.enter_context(tc.tile_pool(name="packr", bufs=2))
            gqp = top.enter_context(tc.tile_pool(name="gqp", bufs=2))
            cbtp = top.enter_context(tc.tile_pool(name="cbtp", bufs=2))
            idxr = top.enter_context(tc.tile_pool(name="idxr", bufs=2))
            rselp = top.enter_context(tc.tile_pool(name="rselp", bufs=2))
            sedp = top.enter_context(tc.tile_pool(name="sedp", bufs=2))
            outp = top.enter_context(tc.tile_pool(name="outp", bufs=2))

            def mlp_fp8(pre, in8, out_f32, htag=""):
                s1, s2 = SCALES[pre]
                shapes = {"enc": [(C, H), (H, H), (H, D)],
                          "nrm": [(C, H), (H, H), (H, D)],
                          "dec": [(D, H), (H, H), (H, C)]}[pre]
                LR = AF.Relu if SIM_RELU else AF.Lrelu
                funcs = {"enc": (LR, LR, AF.Identity),
                         "nrm": (AF.Relu, AF.Relu, AF.Relu),
                         "dec": (LR, LR, None)}[pre]
                alphas = {"enc": 0.01, "dec": 0.01, "nrm": 0.0}[pre]
                scl = [s1 / WS, s2 / (s1 * WS), 1.0 / (s2 * WS)]
                cur = in8
                for li, (ni, no) in enumerate(shapes):
                    npair = ni // 256
                    last = (li == 2)
                    if not last:
                        h = hp.tile([128, no // 256, 2, 512], f8,
                                    tag=f"h{pre}{li}{htag}")
                    for oc in range(no // 128):
                        ps = mps.tile([128, 512], f32, tag="mlp_ps")
                        for cp in range(npair):
                            nc.tensor.matmul(
                                ps[:],
                                w8[f"{pre}{li}"][:, cp, :, oc * 128:(oc + 1) * 128],
                                cur[:, cp], start=(cp == 0),
                                stop=(cp == npair - 1),
                                perf_mode=PM.DoubleRow, skip_group_check=True)
                        if last:
                            nc.scalar.activation(
                                out_f32[:, oc], ps[:], funcs[2] or AF.Identity,
                                bias=bs[f"{pre}{li}"][:, oc:oc + 1],
                                scale=scl[2], alpha=alphas)
                        else:
                            nc.scalar.activation(
                                h[:, oc // 2, oc % 2], ps[:], funcs[li],
                                bias=bs[f"{pre}{li}"][:, oc:oc + 1],
                                scale=scl[li], alpha=alphas)
                    cur = h

            # ============ Phase B: enc + nrm + sen per tt ============
            for tt in range(NTT):
                ts_ = slice(tt * 512, (tt + 1) * 512)
                spk8 = s8p.tile([128, NCP, 2, 512], f8, tag="spk8")
                for cc in range(NCC):
                    fre = frep.tile([128, 512], f32, tag="fre")
                    nc.sync.dma_start(out=fre[:],
                                      in_=dfeat[cc * 128:(cc + 1) * 128, ts_])
                    nc.vector.tensor_tensor(
                        out=spk8[:, cc // 2, cc % 2],
                        in0=fre[:], in1=ld16s[tt][:, cc], op=OP.subtract)
                spk_enc = sep.tile([128, NDC, 512], f32, tag="spk_enc")
                mlp_fp8("enc", spk8, spk_enc)
                nrm8 = s8p.tile([128, NCP, 2, 512], f8, tag="nrm8")
                nc.vector.tensor_copy(
                    nrm8[:].rearrange("p cp i f -> p (cp i) f"),
                    ld16s[tt][:])
                mlp_fp8("nrm", nrm8, norm_v[:, :, ts_])
                rec = sep.tile([128, NDC, 512], f32, tag="rec")
                nc.scalar.add(rec[:], norm_v[:, :, ts_], eps_col[:])
                nc.vector.reciprocal_approx_fast(rec[:], rec[:])
                nc.vector.tensor_tensor(out=r_t[:, :, ts_], in0=spk_enc[:],
                                        in1=rec[:], op=OP.mult)

            # ============ Phase C: RVQ stage-outer ============
            for q in range(NQ):
                cbt = cbtp.tile([128, NDC, BINS], bf16, tag="cbt")
                nc.sync.dma_start(
                    out=cbt[:],
                    in_=dcbT2[q].rearrange("(a p) n -> p a n", p=128))
                rsel = rselp.tile([128, NFT], u16, tag="rsel")
                idxw12 = idxr.tile([128, NFT, 8], i16, tag="idxw12")

                def rvq_update(ctx, q=q):
                    tt, gq16 = ctx
                    ts_ = slice(tt * 512, (tt + 1) * 512)
                    nc.vector.tensor_tensor(out=qs[:, :, ts_],
                                            in0=qs[:, :, ts_],
                                            in1=gq16[:], op=OP.add)
                    if q < NQ - 1:
                        nc.vector.tensor_tensor(out=r_t[:, :, ts_],
                                                in0=r_t[:, :, ts_],
                                                in1=gq16[:], op=OP.subtract)

                pend = None
                for tt in range(NTT):
                    for ftl in range(4):
                        ft = 4 * tt + ftl
                        fs = slice(ft * 128, (ft + 1) * 128)
                        sc = scp.tile([128, BINS], bf16, tag="sc")
                        for half in range(2):
                            hs = slice(half * 512, (half + 1) * 512)
                            psq = rps.tile([128, 512], f32, tag="psq")
                            for dc in range(NDC):
                                nc.tensor.matmul(psq[:], r_t[:, dc, fs],
                                                 cbt[:, dc, hs],
                                                 start=(dc == 0),
                                                 stop=(dc == NDC - 1),
                                                 skip_group_check=True)
                            nc.scalar.activation(
                                sc[:, hs], psq[:], AF.Identity)
                        m8 = packr.tile([128, 8], bf16, tag="m8")
                        nc.vector.max(out=m8[:], in_=sc[:])
                        mi = packr.tile([128, 8], u16, tag="mi")
                        nc.vector.max_index(mi[:], m8[:], sc[:])
                        nc.vector.tensor_copy(rsel[:, ft:ft + 1], mi[:, 0:1])
                    wrap_cols(idxw12[:, 4 * tt:4 * tt + 4],
                              rsel[:, 4 * tt:4 * tt + 4], 4)
                    gq16 = gqp.tile([128, NDC, 512], bf16, tag="gq16")
                    nc.gpsimd.dma_gather(
                        out_ap=gq16[:], in_ap=dcb16[q],
                        idxs_ap=idxw12[:, 4 * tt:4 * tt + 4], num_idxs=512,
                        num_idxs_reg=512, elem_size=D, transpose=True)
                    if pend is not None:
                        rvq_update(pend)
                    pend = (tt, gq16)
                rvq_update(pend)
                if DEBUG:
                    nc.sync.dma_start(out=ddbg_rsel[q], in_=rsel[:])

            # ============ Phase D: sed + dec + out per tt ============
            s1, s2 = SCALES["dec"]
            for tt in range(NTT):
                ts_ = slice(tt * 512, (tt + 1) * 512)
                sed8 = sedp.tile([128, 1, 2, 512], f8, tag="sed8")
                nc.vector.tensor_tensor(out=sed8[:, 0], in0=qs[:, :, ts_],
                                        in1=norm_v[:, :, ts_], op=OP.mult)
                hd0 = hp.tile([128, 2, 2, 512], f8, tag="hd0")
                for oc in range(NHC):
                    ps = mps.tile([128, 512], f32, tag="mlp_ps")
                    nc.tensor.matmul(ps[:],
                                     w8["dec0"][:, 0, :, oc * 128:(oc + 1) * 128],
                                     sed8[:, 0], start=True, stop=True,
                                     perf_mode=PM.DoubleRow,
                                     skip_group_check=True)
                    nc.scalar.activation(hd0[:, oc // 2, oc % 2], ps[:],
                                         AF.Relu if SIM_RELU else AF.Lrelu,
                                         bias=bs["dec0"][:, oc:oc + 1],
                                         scale=s1 / WS, alpha=0.01)
                hd1 = hp.tile([128, 2, 2, 512], f8, tag="hd1")
                for oc in range(NHC):
                    ps = mps.tile([128, 512], f32, tag="mlp_ps")
                    for cp in range(2):
                        nc.tensor.matmul(
                            ps[:], w8["dec1"][:, cp, :, oc * 128:(oc + 1) * 128],
                            hd0[:, cp], start=(cp == 0), stop=(cp == 1),
                            perf_mode=PM.DoubleRow, skip_group_check=True)
                    nc.scalar.activation(hd1[:, oc // 2, oc % 2], ps[:],
                                         AF.Relu if SIM_RELU else AF.Lrelu,
                                         bias=bs["dec1"][:, oc:oc + 1],
                                         scale=s2 / (s1 * WS), alpha=0.01)
                for ftl in range(4):
                    ft = tt * 4 + ftl
                    fs = slice(ft * 128, (ft + 1) * 128)
                    fsl = slice(ftl * 128, (ftl + 1) * 128)
                    for half in range(2):
                        cs = slice(half * 512, (half + 1) * 512)
                        ps = mps.tile([128, 512], f32, tag="mlp_ps")
                        for cp in range(2):
                            nc.tensor.matmul(
                                ps[:], hd1[:, cp, :, fsl],
                                w8["dec2"][:, cp, :, cs],
                                start=(cp == 0), stop=(cp == 1),
                                perf_mode=PM.DoubleRow, skip_group_check=True)
                        ot = outp.tile([128, 512], f32, tag="ot")
                        nc.vector.scalar_tensor_tensor(
                            out=ot[:], in0=ps[:], scalar=1.0 / (s2 * WS),
                            in1=linF[:, ft, cs], op0=OP.mult, op1=OP.add)
                        nc.sync.dma_start(out=dout[fs, cs], in_=ot[:])

    nc.compile()
    return nc


def _get_program():
    global _prog
    if _prog is None:
        _prog = _build_program()
    return _prog


def _host_prep(inputs):
    fp8 = ml_dtypes.float8_e4m3fn
    bf = ml_dtypes.bfloat16
    g = lambda k: np.ascontiguousarray(np.asarray(inputs[k], dtype=np.float32))
    feature = g('feature')
    centroid = g('centroid')
    codebooks = g('codebooks')

    cent_pair = np.ascontiguousarray(
        (2.0 * centroid.T).reshape(NCP, 2, 128, K).transpose(2, 0, 1, 3)
    ).astype(fp8)
    cn = (centroid.astype(np.float64) ** 2).sum(1)
    cent_ext = np.zeros((K, 1088), dtype=np.float32)
    cent_ext[:, :C] = centroid
    cent_ext[:, C] = cn.astype(np.float32)
    combo_row = (np.round(CSHIFT - cn) + np.arange(K) * 2.0 ** -13
                 ).astype(np.float32)
    combo = np.ascontiguousarray(np.broadcast_to(combo_row[None, :], (128, K)))

    shared = {
        "cent8": cent_pair,
        "combo": combo,
        "cent_ext": cent_ext,
        "cent_ob": (centroid + g("dec_b2")[None, :]).astype(bf),
        "cent16": centroid.astype(bf),
        "cbT2": np.ascontiguousarray(
            2.0 * codebooks.transpose(0, 2, 1)).astype(bf),
        "cb16": codebooks.astype(bf),
    }
    for pre in ("enc", "nrm", "dec"):
        s1, s2 = SCALES[pre]
        bscale = {0: s1, 1: s2, 2: 1.0}
        for i in range(3):
            w = g(f"{pre}_w{i}")
            ni, no = w.shape
            shared[f"{pre}_w8{i}"] = np.ascontiguousarray(
                (w * WS).reshape(ni // 256, 2, 128, no).transpose(2, 0, 1, 3)
            ).astype(fp8)
            shared[f"{pre}_bs{i}"] = np.ascontiguousarray(
                g(f"{pre}_b{i}") * bscale[i])

    in_maps = []
    for b in range(B):
        m = dict(shared)
        feats = np.zeros((C, TPAD), dtype=np.float32)
        feats[:, :T] = feature[b]
        m["feat"] = feats
        m["featT"] = np.ascontiguousarray(feats.T)
        m["feat8"] = np.ascontiguousarray(
            feats.reshape(NCP, 2, 128, TPAD).transpose(2, 0, 1, 3)).astype(fp8)
        in_maps.append(m)
    return in_maps


def kernel(**inputs):
    global LAST_RESULT
    from concourse.bass_utils import run_bass_kernel_spmd
    nc = _get_program()
    in_maps = _host_prep(inputs)
    kwargs = {}
    if TRACE:
        try:
            from ntff_shim import install_ntff_hook
            install_ntff_hook()
            kwargs["trace"] = True
        except Exception:
            pass
    res = run_bass_kernel_spmd(nc, in_maps, core_ids=list(range(B)), **kwargs)
    LAST_RESULT = res
    out = np.empty((B, C, T), dtype=np.float32)
    for b in range(B):
        out[b] = res.results[b]["out"][:T].T
    return out
